# revision 19
# baseline (speedup 1.0000x reference)
"""Trainium2 Bass kernel for nn_ContrastLoss (band-limited PSD contrastive loss).

Math notes (all exact identities, not approximations):
  - reference subtracts the per-window mean, but integer-frequency DFT bins
    23..102 are orthogonal to DC, so mean subtraction is a no-op on the band.
  - the band PSD is normalized per window (band / band.sum()), so the
    reference's 1/DELTA_T rfft scaling cancels; raw |X_f|^2 suffices.
  - rfft band bins are two real matmuls: X_f = seg @ cos_f, seg @ sin_f.

Device work per core (8-way channel sharding, 1024 windows/core):
  Inputs ship as fp8_e4m3 (loss error ~1e-4 vs the 2e-2 gate). The DFT basis
  is the stationary operand: per 256-sample k-pair, one DoubleRow fp8 matmul
  contracts 256 time samples for 512 windows at once. 8 weight loads +
  16 matmuls accumulate PSUM [80f, 512win] x {cos,sin} x {video0, video1};
  DVE/GpSimd square + fold into band [80, 1024] bf16, two overlapped DMAs out.
Host: window gather + fp8 shard prep; row-sums, normalization and the
closed-form pairwise-MSE scalars in float64 (cheap: 8x1024x80 values).
"""

import sys

import numpy as np

if "/opt/trn_rl_repo" not in sys.path:
    sys.path.insert(0, "/opt/trn_rl_repo")

import ml_dtypes

B = 2
C = 256
T = 8192
K = 16
DT = 1024
NCORES = 8
CLOC = C // NCORES          # channels per core
SEGS = B * CLOC * K         # windows per core = 1024
F_LO, F_HI = 23, 103        # band bins [23, 102]
NF = F_HI - F_LO            # 80
NW = 2 * NF                 # 160 (cos || sin)
KP = DT // 256              # 4 k-pairs (256 time samples each, 2x128 rows)
NH = 2                      # window halves (0..511 = video 0, 512.. = video 1)
HW_ = SEGS // NH            # 512 windows per half
N_TOT = C * K               # 4096 windows per video

FP8 = ml_dtypes.float8_e4m3


def _dft_basis():
    t = np.arange(DT, dtype=np.float64)
    f = np.arange(F_LO, F_HI, dtype=np.float64)
    ang = 2.0 * np.pi * np.outer(t, f) / DT
    w = np.concatenate([np.cos(ang), np.sin(ang)], axis=1)   # [DT, NW]
    # [p, pair, i, j] = W[256*pair + 128*i + p, j]
    return np.ascontiguousarray(
        w.reshape(KP, 2, 128, NW).transpose(2, 0, 1, 3)
    ).astype(FP8)


_W_FP8 = _dft_basis()
_NC = None


def _build_nc():
    import concourse.mybir as mybir
    import concourse.tile as tile
    from concourse import bacc

    nc = bacc.Bacc(
        "TRN2",
        target_bir_lowering=False,
        debug=False,
        enable_asserts=True,
        num_devices=NCORES,
    )
    f32 = mybir.dt.float32
    bf16 = mybir.dt.bfloat16
    fp8 = mybir.dt.float8e4
    segs_d = nc.dram_tensor("segs", [KP, 128, 2, SEGS], fp8, kind="ExternalInput").ap()
    w_d = nc.dram_tensor("w", [128, KP, 2, NW], fp8, kind="ExternalInput").ap()
    # out[h] = [NF, cos|sin, HW_] bf16 raw DFT parts; host squares and folds.
    out_d = nc.dram_tensor("out", [NH, NF, 2, HW_], bf16, kind="ExternalOutput").ap()

    dr = mybir.MatmulPerfMode.DoubleRow

    with tile.TileContext(nc) as tc:
        with (
            tc.tile_pool(name="sb", bufs=1) as sb,
            tc.tile_pool(name="psum", bufs=1, space="PSUM") as psump,
        ):
            # PE warm-up: dummy fp8 DoubleRow matmuls with no data deps keep
            # the PE busy through the DMA ramp so the HAM clock gate is high
            # (and stays high) when the real matmuls arrive.
            scratch = sb.tile([128, 2, HW_], fp8)
            nc.gpsimd.memset(scratch[:], 0.0)
            warm_ps = psump.tile([128, HW_], f32, tag="warm")
            NWARM = 6
            for i in range(NWARM):
                nc.tensor.matmul(
                    warm_ps[:],
                    scratch[:, :, :128],
                    scratch[:],
                    start=(i == 0),
                    stop=(i == NWARM - 1),
                    perf_mode=dr,
                )

            # Input stream over both HWDGE queues. w goes first on sync (all
            # matmuls need it); seg k-pairs alternate scalar/sync.
            w_t = sb.tile([128, KP, 2, NW], fp8)
            nc.sync.dma_start(w_t[:], w_d[:])
            seg_t = []
            for p in range(KP):
                st = sb.tile([128, 2, SEGS], fp8, tag=f"seg{p}")
                eng = nc.scalar if p % 2 == 0 else nc.sync
                eng.dma_start(st[:], segs_d[p])
                seg_t.append(st)

            # separate PSUM tiles per (half, cos/sin): keeps the matmul
            # pipeline free of same-tile serialization
            pst = [[psump.tile([NF, HW_], f32, tag=f"ps{h}{cs}", name=f"ps{h}{cs}")
                    for cs in range(2)] for h in range(NH)]
            for p in range(KP):
                # last pair: finish half 0 first so its casts + output DMA
                # overlap the half-1 matmuls
                if p == KP - 1:
                    order = [(0, 0), (0, 1), (1, 0), (1, 1)]
                else:
                    order = [(0, 0), (1, 0), (0, 1), (1, 1)]
                for h, cs in order:
                    nc.tensor.matmul(
                        pst[h][cs][:],
                        w_t[:, p, :, cs * NF:(cs + 1) * NF],
                        seg_t[p][:, :, h * HW_:(h + 1) * HW_],
                        start=(p == 0),
                        stop=(p == KP - 1),
                        perf_mode=dr,
                    )

            outbuf = sb.tile([NF, NH, 2, HW_], bf16)
            for h in range(NH):
                nc.vector.tensor_copy(outbuf[:, h, 0, :], pst[h][0][:])
                nc.vector.tensor_copy(outbuf[:, h, 1, :], pst[h][1][:])
                eng = nc.scalar if h == 0 else nc.sync
                eng.dma_start(out_d[h], outbuf[:, h, :, :])

    nc.compile()
    return nc


def _get_nc():
    global _NC
    if _NC is None:
        _NC = _build_nc()
    return _NC


def _prep_in_maps(model_output, offsets):
    model_output = np.ascontiguousarray(model_output, dtype=np.float32)
    off = np.asarray(offsets, dtype=np.int64)
    sw = np.lib.stride_tricks.sliding_window_view(model_output, DT, axis=-1)
    bi = np.arange(B)[:, None, None]
    ci = np.arange(C)[None, :, None]
    seg = sw[bi, ci, off]                       # [B, C, K, DT] f32
    in_maps = []
    for c in range(NCORES):
        sl = seg[:, c * CLOC:(c + 1) * CLOC].reshape(SEGS, DT)
        # [pair, p, i, s] = seg(window s, time 256*pair + 128*i + p)
        arr = np.ascontiguousarray(
            sl.reshape(SEGS, KP, 2, 128).transpose(1, 3, 2, 0)
        ).astype(FP8)
        in_maps.append({"segs": arr, "w": _W_FP8})
    return in_maps


def _finish(results):
    s = np.zeros((B, NF), dtype=np.float64)
    sq = np.zeros(B, dtype=np.float64)
    for c in range(NCORES):
        x = results[c]["out"].astype(np.float64)        # [NH, NF, 2, HW_]
        band = x[:, :, 0, :] ** 2 + x[:, :, 1, :] ** 2  # [NH, NF, HW_]
        for h in range(NH):
            v = band[h]                                  # [NF, HW_]; half h = video h
            rs = v.sum(axis=0)                           # [HW_]
            s[h] += (v / rs).sum(axis=1)
            sq[h] += ((v * v).sum(axis=0) / (rs * rs)).sum()
    n = float(N_TOT)
    pos_per = (2.0 * n * sq - 2.0 * (s * s).sum(-1)) / NF / (n * n - n)
    pos = (pos_per[0] + pos_per[1]) / 2.0
    neg = -(n * sq[0] + n * sq[1] - 2.0 * float(np.dot(s[0], s[1]))) / NF / (n * n)
    return np.float32(pos + neg), np.float32(pos), np.float32(neg)


def kernel(model_output, offsets):
    from concourse.bass_utils import run_bass_kernel_spmd

    nc = _get_nc()
    in_maps = _prep_in_maps(model_output, offsets)
    res = run_bass_kernel_spmd(nc, in_maps, core_ids=list(range(NCORES)))
    return _finish(res.results)


# revision 24
# speedup vs baseline: 1.0727x; 1.0727x over previous
"""Trainium2 Bass kernel for nn_ContrastLoss (band-limited PSD contrastive loss).

Math notes (all exact identities, not approximations):
  - reference subtracts the per-window mean, but integer-frequency DFT bins
    23..102 are orthogonal to DC, so mean subtraction is a no-op on the band.
  - the band PSD is normalized per window (band / band.sum()), so the
    reference's 1/DELTA_T rfft scaling cancels; raw |X_f|^2 suffices.
  - rfft band bins are two real matmuls: X_f = seg @ cos_f, seg @ sin_f.

Device work per core (8-way channel sharding, 1024 windows/core):
  Inputs ship as fp8_e4m3 (loss error ~1e-4 vs the 2e-2 gate). The DFT basis
  is the stationary operand: per 256-sample k-pair, one DoubleRow fp8 matmul
  contracts 256 time samples for 512 windows at once; 16 matmuls accumulate
  PSUM [80f, (cos|sin), 512win] per video-half. One DVE cast per half moves
  raw X to bf16; two overlapped DMAs ship it out. Host squares/normalizes.
Host: window gather + fp8 shard prep; squares, row-sums and the closed-form
pairwise-MSE scalars in float64 (cheap: 8x1024x160 values).
"""

import sys

import numpy as np

if "/opt/trn_rl_repo" not in sys.path:
    sys.path.insert(0, "/opt/trn_rl_repo")

import ml_dtypes

B = 2
C = 256
T = 8192
K = 16
DT = 1024
NCORES = 8
CLOC = C // NCORES          # channels per core
SEGS = B * CLOC * K         # windows per core = 1024
F_LO, F_HI = 23, 103        # band bins [23, 102]
NF = F_HI - F_LO            # 80
NW = 2 * NF                 # 160 (cos || sin)
KP = DT // 256              # 4 k-pairs (256 time samples each, 2x128 rows)
NH = 2                      # window halves (0..511 = video 0, 512.. = video 1)
HW_ = SEGS // NH            # 512 windows per half
N_TOT = C * K               # 4096 windows per video

FP8 = ml_dtypes.float8_e4m3


def _dft_basis():
    t = np.arange(DT, dtype=np.float64)
    f = np.arange(F_LO, F_HI, dtype=np.float64)
    ang = 2.0 * np.pi * np.outer(t, f) / DT
    w = np.concatenate([np.cos(ang), np.sin(ang)], axis=1)   # [DT, NW]
    # [p, pair, i, j] = W[256*pair + 128*i + p, j]
    return np.ascontiguousarray(
        w.reshape(KP, 2, 128, NW).transpose(2, 0, 1, 3)
    ).astype(FP8)


_W_FP8 = _dft_basis()
_NC = None


def _build_nc():
    import concourse.mybir as mybir
    import concourse.tile as tile
    from concourse import bacc

    nc = bacc.Bacc(
        "TRN2",
        target_bir_lowering=False,
        debug=False,
        enable_asserts=True,
        num_devices=NCORES,
    )
    f32 = mybir.dt.float32
    bf16 = mybir.dt.bfloat16
    fp8 = mybir.dt.float8e4
    # seg pair-chunks: [pair, p, i, win]; pairs 0,1 land first (scalar queue)
    seg0_d = nc.dram_tensor("seg0", [128, 2, SEGS], fp8, kind="ExternalInput").ap()
    seg1_d = nc.dram_tensor("seg1", [128, 2, SEGS], fp8, kind="ExternalInput").ap()
    seg23_d = nc.dram_tensor("seg23", [128, 2, 2, SEGS], fp8, kind="ExternalInput").ap()
    w_d = nc.dram_tensor("w", [128, KP, 2, NW], fp8, kind="ExternalInput").ap()
    # out[h] = [NF, cos|sin, HW_] bf16 raw DFT parts; host squares and folds.
    out_d = nc.dram_tensor("out", [NH, NF, 2, HW_], bf16, kind="ExternalOutput").ap()

    dr = mybir.MatmulPerfMode.DoubleRow

    with tile.TileContext(nc) as tc:
        with (
            tc.tile_pool(name="sb", bufs=1) as sb,
            tc.tile_pool(name="psum", bufs=1, space="PSUM") as psump,
        ):
            # PE warm-up: dummy fp8 DoubleRow matmuls with no data deps keep
            # the PE busy through the DMA ramp so the HAM clock gate is high
            # when the real matmuls arrive.
            scratch = sb.tile([128, 2, HW_], fp8)
            nc.gpsimd.memset(scratch[:], 0.0)
            warm_ps = psump.tile([128, HW_], f32, tag="warm")
            NWARM = 5
            for i in range(NWARM):
                nc.tensor.matmul(
                    warm_ps[:],
                    scratch[:, :, :128],
                    scratch[:],
                    start=(i == 0),
                    stop=(i == NWARM - 1),
                    perf_mode=dr,
                )

            # Input stream over both HWDGE queues, ordered so pair 0 and the
            # basis land first: scalar: seg0, seg1; sync: w, seg23.
            w_t = sb.tile([128, KP, 2, NW], fp8)
            nc.sync.dma_start(w_t[:], w_d[:])
            sega = sb.tile([128, 2, SEGS], fp8, tag="sega")
            nc.scalar.dma_start(sega[:], seg0_d[:])
            segb = sb.tile([128, 2, SEGS], fp8, tag="segb")
            nc.scalar.dma_start(segb[:], seg1_d[:])
            segc = sb.tile([128, 2, 2, SEGS], fp8, tag="segc")
            nc.sync.dma_start(segc[:], seg23_d[:])
            seg_view = [sega, segb, None, None]

            # ps[h]: [NF, (cos|sin), HW_] f32 spanning two PSUM banks, so one
            # DVE cast covers a whole half's raw output.
            ps = [psump.tile([NF, 2, HW_], f32, tag=f"ps{h}", name=f"ps{h}")
                  for h in range(NH)]
            for p in range(KP):
                # last pair: finish half 0 completely first so its cast +
                # output DMA overlap the half-1 matmuls; otherwise share each
                # LDWEIGHTS between both halves.
                if p == KP - 1:
                    order = [(0, 0), (0, 1), (1, 0), (1, 1)]
                else:
                    order = [(0, 0), (1, 0), (0, 1), (1, 1)]
                for h, cs in order:
                    st = seg_view[p]
                    rhs = (st[:, :, h * HW_:(h + 1) * HW_] if st is not None
                           else segc[:, p - 2, :, h * HW_:(h + 1) * HW_])
                    nc.tensor.matmul(
                        ps[h][:, cs, :],
                        w_t[:, p, :, cs * NF:(cs + 1) * NF],
                        rhs,
                        start=(p == 0),
                        stop=(p == KP - 1),
                        perf_mode=dr,
                    )

            outbuf = sb.tile([NF, NH, 2, HW_], bf16)
            for h in range(NH):
                nc.vector.tensor_copy(outbuf[:, h, :, :], ps[h][:])
                eng = nc.scalar if h == 0 else nc.sync
                eng.dma_start(out_d[h], outbuf[:, h, :, :])

    nc.compile()
    return nc


def _get_nc():
    global _NC
    if _NC is None:
        _NC = _build_nc()
    return _NC


def _prep_in_maps(model_output, offsets):
    model_output = np.ascontiguousarray(model_output, dtype=np.float32)
    off = np.asarray(offsets, dtype=np.int64)
    sw = np.lib.stride_tricks.sliding_window_view(model_output, DT, axis=-1)
    bi = np.arange(B)[:, None, None]
    ci = np.arange(C)[None, :, None]
    seg = sw[bi, ci, off]                       # [B, C, K, DT] f32
    in_maps = []
    for c in range(NCORES):
        sl = seg[:, c * CLOC:(c + 1) * CLOC].reshape(SEGS, DT)
        # [pair, p, i, s] = seg(window s, time 256*pair + 128*i + p)
        arr = np.ascontiguousarray(
            sl.reshape(SEGS, KP, 2, 128).transpose(1, 3, 2, 0)
        ).astype(FP8)
        in_maps.append({
            "seg0": arr[0], "seg1": arr[1],
            "seg23": np.ascontiguousarray(arr[2:4].transpose(1, 0, 2, 3)),
            "w": _W_FP8,
        })
    return in_maps


def _finish(results):
    s = np.zeros((B, NF), dtype=np.float64)
    sq = np.zeros(B, dtype=np.float64)
    for c in range(NCORES):
        x = results[c]["out"].astype(np.float64)        # [NH, NF, 2, HW_]
        band = x[:, :, 0, :] ** 2 + x[:, :, 1, :] ** 2  # [NH, NF, HW_]
        for h in range(NH):
            v = band[h]                                  # [NF, HW_]; half h = video h
            rs = v.sum(axis=0)                           # [HW_]
            s[h] += (v / rs).sum(axis=1)
            sq[h] += ((v * v).sum(axis=0) / (rs * rs)).sum()
    n = float(N_TOT)
    pos_per = (2.0 * n * sq - 2.0 * (s * s).sum(-1)) / NF / (n * n - n)
    pos = (pos_per[0] + pos_per[1]) / 2.0
    neg = -(n * sq[0] + n * sq[1] - 2.0 * float(np.dot(s[0], s[1]))) / NF / (n * n)
    return np.float32(pos + neg), np.float32(pos), np.float32(neg)


def kernel(model_output, offsets):
    from concourse.bass_utils import run_bass_kernel_spmd

    nc = _get_nc()
    in_maps = _prep_in_maps(model_output, offsets)
    res = run_bass_kernel_spmd(nc, in_maps, core_ids=list(range(NCORES)))
    return _finish(res.results)
